# revision 7
# baseline (speedup 1.0000x reference)
"""Trainium2 Bass kernel for nn_ESBN_77352361001553 (scatter_memory).

Math being computed (see the reference's own faithfulness note): the conv
encoder output is dead code, and the LSTM input is constant zeros, so the
gate pre-activation contribution from the input is the constant bih + bhh
for every step and every batch element. Every batch row therefore follows
the identical 16-step, 512-dim LSTM trajectory from zero state, and the
(16, 1024, 4) output is out_t = Wo @ h_t + bo broadcast across batch.

Sharding: pure data parallelism over the batch dim — each of the 8 cores
owns a 128-wide batch shard. Each core runs the recurrence on-chip:
 - gates matvec on the PE as 64 (LDWEIGHTS, MATMUL N=1) pairs per step in
   fp16 (FWL fast-weight-load path), accumulating h/c-layout [128, 16]
   gate columns in PSUM,
 - sigmoid/tanh on the ACT engine, state updates on the DVE,
 - output head as 4 accumulating matmuls, then a broadcast over the batch
   shard via a ones-vector matmul, and one DMA of the (16, 128, 4) shard.
Host code only re-lays-out the tiny weights and concatenates shards.
"""

import os

import numpy as np

T = 16
HID = 512
N_CORES = 8
BSH = 128  # batch shard per core

_BUILT = {}
last_results = None  # BassKernelResults of the most recent run (for tooling)


def _ensure_ntff_hook():
    """Register the axon NTFF profiling hook if the container lacks
    antenv.axon_hooks (slim boot). Mirrors trn_boot._ntff_profile_via_ctypes."""
    import contextlib
    import ctypes
    import sys
    import types

    try:
        from antenv.axon_hooks import get_axon_ntff_profile_hook  # noqa: F401

        return
    except ImportError:
        pass

    so_path = "/opt/axon/libaxon_pjrt.so"
    hook = None
    if os.path.exists(so_path):
        lib = ctypes.CDLL(so_path)
        if hasattr(lib, "axon_start_nrt_profile"):
            lib.axon_start_nrt_profile.argtypes = [
                ctypes.POINTER(ctypes.c_int64),
                ctypes.c_size_t,
            ]
            lib.axon_start_nrt_profile.restype = ctypes.c_int64
            lib.axon_stop_nrt_profile.argtypes = [ctypes.c_char_p]
            lib.axon_stop_nrt_profile.restype = ctypes.c_int64

            @contextlib.contextmanager
            def _hook(output_dir, device_ids):
                import jax

                jax.devices()  # force PJRT init so the .so's client exists
                if device_ids:
                    ids = (ctypes.c_int64 * len(device_ids))(*device_ids)
                    rc = lib.axon_start_nrt_profile(ids, len(device_ids))
                else:
                    rc = lib.axon_start_nrt_profile(None, 0)
                if rc != 0:
                    raise RuntimeError(f"axon_start_nrt_profile rc={rc}")
                try:
                    yield
                finally:
                    n = lib.axon_stop_nrt_profile(str(output_dir).encode())
                    print(f"ntff profile: {n} file(s) -> {output_dir}", file=sys.stderr)

            hook = _hook

    mod = types.ModuleType("antenv.axon_hooks")
    mod.get_axon_ntff_profile_hook = lambda: hook
    mod.set_axon_ntff_profile_hook = lambda h: None
    import antenv

    antenv.axon_hooks = mod
    sys.modules["antenv.axon_hooks"] = mod


def _build(nsteps=T):
    """Assemble the Bass module (one NeuronCore program, SPMD across 8)."""
    import concourse.bacc as bacc
    import concourse.mybir as mybir
    from concourse import tile

    f32 = mybir.dt.float32
    f16 = mybir.dt.float16
    AF = mybir.ActivationFunctionType

    nc = bacc.Bacc("TRN2", target_bir_lowering=False, debug=False)

    wT_d = nc.dram_tensor("wT", [128, 8192], f16, kind="ExternalInput")
    cst_d = nc.dram_tensor("cst", [128, 16], f32, kind="ExternalInput")
    woT_d = nc.dram_tensor("woT", [128, 16], f16, kind="ExternalInput")
    bo_d = nc.dram_tensor("bo64", [1, 64], f32, kind="ExternalInput")
    out_d = nc.dram_tensor("out", [T, BSH, 4], f32, kind="ExternalOutput")

    with tile.TileContext(nc) as tc:
        with (
            tc.tile_pool(name="w", bufs=1) as wp,
            tc.tile_pool(name="st", bufs=1) as sp,
            tc.tile_pool(name="tmp", bufs=2) as tp,
            tc.tile_pool(name="ps", bufs=1, space="PSUM") as pp,
        ):
            wT = wp.tile([128, 8192], f16)
            cst = sp.tile([128, 16], f32)
            woT = sp.tile([128, 16], f16)
            bo64 = sp.tile([1, 64], f32)
            nc.sync.dma_start(wT[:], wT_d[:])
            nc.sync.dma_start(cst[:], cst_d[:])
            nc.sync.dma_start(woT[:], woT_d[:])
            nc.sync.dma_start(bo64[:], bo_d[:])

            # "Landing" ops: give each DMA-loaded tensor a first consumer per
            # engine with no other cross-engine deps, so no downstream
            # instruction ever needs more than one sync-wait slot.
            land = tp.tile([128, 1], f32, tag="land")
            nc.vector.tensor_copy(land[:], cst[:, 0:1])
            land2 = tp.tile([1, 1], f32, tag="land2")
            nc.vector.tensor_copy(land2[:], bo64[:, 0:1])
            psd = pp.tile([128, 1], f32, tag="dummy")
            nc.tensor.matmul(psd[:], wT[:, 0:128], woT[:, 0:1], start=True, stop=True)

            # h_t history, fp16, column 4t+ko holds h_t[ko*128 + p]
            hs = sp.tile([128, 4 * T], f16)
            cx = sp.tile([128, 4], f32)
            psg = pp.tile([128, 16], f32, tag="gates")

            def nonlin(gsrc, t):
                # gsrc [128, 16]: gate pre-activations, col blocks i|f|o|g
                sig = tp.tile([128, 12], f32, tag="sig")
                tg = tp.tile([128, 4], f32, tag="tg")
                th = tp.tile([128, 4], f32, tag="th")
                nc.scalar.activation(sig[:], gsrc[:, 0:12], AF.Sigmoid)
                nc.scalar.activation(tg[:], gsrc[:, 12:16], AF.Tanh)
                if t == 0:
                    # c starts at zero: c = sig_i * tanh_g
                    nc.vector.tensor_mul(cx[:], sig[:, 0:4], tg[:])
                else:
                    t1 = tp.tile([128, 4], f32, tag="t1")
                    nc.vector.tensor_mul(t1[:], sig[:, 0:4], tg[:])
                    nc.vector.tensor_mul(cx[:], sig[:, 4:8], cx[:])
                    nc.vector.tensor_add(cx[:], cx[:], t1[:])
                nc.scalar.activation(th[:], cx[:], AF.Tanh)
                nc.vector.tensor_mul(hs[:, 4 * t : 4 * t + 4], sig[:, 8:12], th[:])

            nonlin(cst, 0)  # step 0: gates == constant, no matvec needed

            for t in range(1, nsteps):
                for jo in range(16):
                    for ko in range(4):
                        nc.tensor.matmul(
                            psg[:, jo : jo + 1],
                            wT[:, ko * 2048 + jo * 128 : ko * 2048 + jo * 128 + 128],
                            hs[:, 4 * (t - 1) + ko : 4 * (t - 1) + ko + 1],
                            start=(ko == 0),
                            stop=(ko == 3),
                        )
                g = tp.tile([128, 16], f32, tag="g")
                nc.vector.tensor_add(g[:], psg[:], cst[:])
                nonlin(g, t)

            # head: hps[t, d] = sum_k Wo[d, k] h_t[k]
            hps = pp.tile([16, 4], f32, tag="head")
            for ko in range(4):
                nc.tensor.matmul(
                    hps[:],
                    hs[:, ko : ko + 4 * (T - 1) + 1 : 4],  # lhsT [K=128, M=16 steps]
                    woT[:, 4 * ko : 4 * ko + 4],  # rhs [K=128, N=4]
                    start=(ko == 0),
                    stop=(ko == 3),
                )
            head = sp.tile([16, 4], f32)
            nc.vector.tensor_copy(head[:], hps[:])
            # flatten the [16, 4] head onto one partition as [1, 64]
            flat = sp.tile([1, 64], f32)
            nc.sync.dma_start(
                flat[0:1, :].rearrange("a (t d) -> a t d", d=4), head[:]
            )
            nc.vector.tensor_add(flat[:], flat[:], bo64[:])
            # broadcast to all 128 batch partitions: ones[128] ⊗ flat[64]
            ones = sp.tile([1, 128], f32)
            nc.vector.memset(ones[:], 1.0)
            bcp = pp.tile([128, 64], f32, tag="bc")
            nc.tensor.matmul(bcp[:], ones[:], flat[:], start=True, stop=True)
            bc = sp.tile([128, 64], f32)
            nc.vector.tensor_copy(bc[:], bcp[:])
            nc.sync.dma_start(
                out_d.rearrange("t b d -> b t d"),
                bc[:].rearrange("p (t d) -> p t d", d=4),
            )
    nc.compile()
    return nc


def prep_inputs(Whh, bih, bhh, Wo, bo):
    """Host-side weight relayout (all tensors are tiny: <5 MB total)."""
    Whh = np.asarray(Whh, np.float32)
    c = (np.asarray(bih, np.float32) + np.asarray(bhh, np.float32))
    Wo = np.asarray(Wo, np.float32)
    bo = np.asarray(bo, np.float32)
    H = HID
    # reorder gate blocks from torch's i,f,g,o to i,f,o,g so sigmoid gates
    # occupy columns 0:12 and tanh gates columns 12:16
    perm = np.concatenate(
        [np.arange(0, 2 * H), np.arange(3 * H, 4 * H), np.arange(2 * H, 3 * H)]
    )
    Wp = Whh[perm]
    cp = c[perm]
    wT = np.ascontiguousarray(
        Wp.reshape(2048, 4, 128).transpose(2, 1, 0).reshape(128, 8192)
    ).astype(np.float16)
    cst = np.ascontiguousarray(cp.reshape(16, 128).T).astype(np.float32)
    woT = np.ascontiguousarray(
        Wo.reshape(4, 4, 128).transpose(2, 1, 0).reshape(128, 16)
    ).astype(np.float16)
    bo64 = np.tile(bo, T)[None, :].astype(np.float32)
    return {"wT": wT, "cst": cst, "woT": woT, "bo64": bo64}


def kernel(**inputs) -> np.ndarray:
    global last_results
    from concourse.bass_utils import run_bass_kernel_spmd

    if "nc" not in _BUILT:
        _BUILT["nc"] = _build()
    nc = _BUILT["nc"]

    in_map = prep_inputs(
        inputs["Whh"], inputs["bih"], inputs["bhh"], inputs["Wo"], inputs["bo"]
    )
    if os.environ.get("BASS_TRACE"):
        _ensure_ntff_hook()
    in_maps = [dict(in_map) for _ in range(N_CORES)]
    res = run_bass_kernel_spmd(
        nc,
        in_maps,
        core_ids=list(range(N_CORES)),
        trace=bool(os.environ.get("BASS_TRACE")),
    )
    last_results = res
    # gather: concatenate the 8 per-core batch shards
    return np.concatenate([r["out"] for r in res.results], axis=1)
